# Initial kernel scaffold
#
"""Trainium2 Bass kernel for CustomRNN.

Reference computation (all fp32):
    xproj = einsum('bti,ih->bth', x, Wxh) + b_xh          # B,T,HID
    h_{t+1} = tanh(xproj[:,t] + h_t @ Whh + b_hh)         # scan over T
    out = h_T @ Why + b_y                                  # B,NC

Strategy:
  - Batch-parallel over NCORES NeuronCores (no cross-core comms).
  - Per core everything is kept TRANSPOSED: hT[k, b] with hidden on
    partitions, batch on the free dim.  Then both the recurrence
    (Whh-tiles stationary) and the input projection (Wxh-tiles
    stationary) accumulate into the same PSUM tile [128j, BS] and the
    tanh produces hT_{t+1} tiles directly in the layout the next step
    consumes -> zero per-step transposes.
  - x is pre-transposed on the host to [T, IN, B] so the per-timestep
    slab DMAs straight into [i-partition, batch] layout.
  - Matmul operands use float32r (e8m11, fp32 accumulation in PSUM):
    full-rate (1 cycle per moving row) once the moving dim N =
    per-core batch >= 256.  That is why the batch is split over 4
    cores (N=256) rather than 8 (N=128 runs at 1/4 rate): fewer
    cores, 2x faster wall clock.  Operands are pre-rounded to e8m11
    on the host (round-to-nearest-even) which matches the PE input
    rounding exactly, so shipping them as float32r-typed DRAM is
    numerically identical to HW-side rounding.
"""

import os
import numpy as np

B, T, IN, HID, NCLS = 1024, 128, 512, 1024, 10
P = 128
NCORES = int(os.environ.get("RNN_NCORES", "4"))
BS = B // NCORES                 # per-core batch (moving dim N)
JT = HID // P                    # output (M) tiles of h
KT = HID // P                    # contraction tiles over h
IT = IN // P                     # contraction tiles over x
T_STEPS = int(os.environ.get("RNN_T", str(T)))
REPEAT = int(os.environ.get("RNN_REPEAT", "1"))  # timing aid: loop recurrence R times

_CACHE = {}


def _round_e8m11(a):
    """Round fp32 array to e8m11 (float32r) with round-to-nearest-even."""
    u = np.ascontiguousarray(a, dtype=np.float32).view(np.uint32)
    low = u & np.uint32(0xFFF)
    base = u & np.uint32(0xFFFFF000)
    lsb = (u >> np.uint32(12)) & np.uint32(1)
    round_up = (low > 0x800) | ((low == 0x800) & (lsb == 1))
    out = base + (round_up.astype(np.uint32) << np.uint32(12))
    return out.view(np.float32)


def _build_nc():
    from concourse import bacc, mybir

    f32 = mybir.dt.float32
    f32r = mybir.dt.float32r
    nc = bacc.Bacc(
        "TRN2", target_bir_lowering=False, debug=False, num_devices=NCORES
    )

    xT = nc.declare_dram_parameter("xT", [T_STEPS, IN, BS], f32r, isOutput=False)
    wxh = nc.declare_dram_parameter("wxh", [IN, HID], f32r, isOutput=False)
    whh = nc.declare_dram_parameter("whh", [HID, HID], f32r, isOutput=False)
    bias = nc.declare_dram_parameter("bias", [P, JT], f32, isOutput=False)
    why = nc.declare_dram_parameter("why", [HID, NCLS], f32r, isOutput=False)
    b_y = nc.declare_dram_parameter("b_y", [1, NCLS], f32r, isOutput=False)
    ones = nc.declare_dram_parameter("ones", [1, P], f32r, isOutput=False)
    out = nc.declare_dram_parameter("out", [BS, NCLS], f32, isOutput=True)
    return nc, (xT, wxh, whh, bias, why, b_y, ones, out)


def _emit(nc, tensors):
    from contextlib import ExitStack

    import concourse.bass as bass
    import concourse.tile as tile
    from concourse import mybir

    f32 = mybir.dt.float32
    f32r = mybir.dt.float32r
    TANH = mybir.ActivationFunctionType.Tanh
    ts = bass.ts
    xT, wxh, whh, bias, why, b_y, ones, out = tensors

    with tile.TileContext(nc) as tc, ExitStack() as ctx:
        const = ctx.enter_context(tc.tile_pool(name="const", bufs=1))
        xpool = ctx.enter_context(tc.tile_pool(name="x", bufs=8))
        pspool = ctx.enter_context(tc.tile_pool(name="ps", bufs=8, space="PSUM"))
        opool = ctx.enter_context(tc.tile_pool(name="o", bufs=2))

        # --- persistent SBUF: weights, biases, h double buffer ---
        whh_sb = [const.tile([P, HID], f32r, name=f"whh{k}", tag=f"whh{k}") for k in range(KT)]
        wxh_sb = [const.tile([P, HID], f32r, name=f"wxh{i}", tag=f"wxh{i}") for i in range(IT)]
        bias_sb = const.tile([P, JT], f32, name="bias_sb", tag="bias")
        why_sb = [const.tile([P, NCLS], f32r, name=f"why{k}", tag=f"why{k}") for k in range(KT)]
        by_sb = const.tile([1, NCLS], f32r, name="by_sb", tag="by")
        ones_sb = const.tile([1, P], f32r, name="ones_sb", tag="ones")
        h_sb = [
            [const.tile([P, BS], f32r, name=f"h{p}_{k}", tag=f"h{p}_{k}") for k in range(KT)]
            for p in range(2)
        ]

        for k in range(KT):
            nc.sync.dma_start(whh_sb[k][:], whh[ts(k, P), :])
            nc.sync.dma_start(why_sb[k][:], why[ts(k, P), :])
        for i in range(IT):
            nc.sync.dma_start(wxh_sb[i][:], wxh[ts(i, P), :])
        nc.sync.dma_start(bias_sb[:], bias[:, :])
        nc.sync.dma_start(by_sb[:], b_y[:, :])
        nc.sync.dma_start(ones_sb[:], ones[:, :])

        # --- recurrence over timesteps ---
        rep_ctx = tc.For_i(0, REPEAT, 1) if REPEAT > 1 else None
        if rep_ctx is not None:
            rep_ctx.__enter__()
        for t in range(T_STEPS):
            xt = xpool.tile([P, IT * BS], f32r, name="xt", tag="xt")
            for i in range(IT):
                nc.sync.dma_start(
                    xt[:, ts(i, BS)], xT[t, ts(i, P), :]
                )
            cur = h_sb[t % 2]
            nxt = h_sb[(t + 1) % 2]
            # Phase 1: all groups' input projections first — these have no
            # dependency on the previous step's tanh outputs, so they give
            # the PE a 32-matmul head start while ACT/DMA tails drain.
            ps_list = []
            for j in range(JT):
                ps = pspool.tile([P, BS], f32, name="ps", tag="ps")
                ps_list.append(ps)
                for i in range(IT):
                    nc.tensor.matmul(
                        ps[:],
                        wxh_sb[i][:, ts(j, P)],
                        xt[:, ts(i, BS)],
                        start=(i == 0),
                        stop=(t == 0 and i == IT - 1),
                    )
            # Phase 2: recurrence + tanh per group.
            for j in range(JT):
                ps = ps_list[j]
                if t > 0:
                    for k in range(KT):
                        nc.tensor.matmul(
                            ps[:],
                            whh_sb[k][:, ts(j, P)],
                            cur[k][:],
                            start=False,
                            stop=(k == KT - 1),
                        )
                nc.scalar.activation(
                    nxt[j][:], ps[:], TANH, bias=bias_sb[:, j : j + 1]
                )

        if rep_ctx is not None:
            rep_ctx.__exit__(None, None, None)

        # --- logits: out[b, n] = h_T[b, :] @ Why + b_y ---
        hT = h_sb[T_STEPS % 2]
        for bh in range(BS // P):
            ps = pspool.tile([P, NCLS], f32, name="ps", tag="ps")
            for k in range(KT):
                nc.tensor.matmul(
                    ps[:],
                    hT[k][:, ts(bh, P)],
                    why_sb[k][:],
                    start=(k == 0),
                    stop=False,
                )
            # broadcast-add b_y via a K=1 matmul: ones[1,P].T @ b_y[1,N]
            nc.tensor.matmul(
                ps[:], ones_sb[:], by_sb[:], start=False, stop=True
            )
            osb = opool.tile([P, NCLS], f32, name="osb", tag="osb")
            nc.vector.tensor_copy(osb[:], ps[:])
            nc.sync.dma_start(out[ts(bh, P), :], osb[:])


def _get_program():
    if "nc" not in _CACHE:
        nc, tensors = _build_nc()
        _emit(nc, tensors)
        nc.compile()
        _CACHE["nc"] = nc
    return _CACHE["nc"]


def _make_in_maps(x, Wxh, b_xh, Whh, b_hh, Why, b_y):
    x = np.asarray(x, dtype=np.float32)
    Wxh_r = _round_e8m11(Wxh)
    Whh_r = _round_e8m11(Whh)
    Why_r = _round_e8m11(Why)
    b_y_r = _round_e8m11(np.asarray(b_y, np.float32).reshape(1, NCLS))
    bias = (np.asarray(b_xh, np.float32) + np.asarray(b_hh, np.float32))
    bias2d = np.ascontiguousarray(bias.reshape(JT, P).T)
    ones_row = np.ones((1, P), dtype=np.float32)

    # host-side layout marshaling: [B,T,IN] -> [T,IN,B], shard batch
    xT_full = _round_e8m11(np.transpose(x[:, :T_STEPS, :], (1, 2, 0)))

    in_maps = []
    for c in range(NCORES):
        in_maps.append(
            {
                "xT": np.ascontiguousarray(xT_full[:, :, c * BS : (c + 1) * BS]),
                "wxh": Wxh_r,
                "whh": Whh_r,
                "bias": bias2d,
                "why": Why_r,
                "b_y": b_y_r,
                "ones": ones_row,
            }
        )
    return in_maps


def kernel(x, Wxh, b_xh, Whh, b_hh, Why, b_y):
    from concourse.bass_utils import run_bass_kernel_spmd

    in_maps = _make_in_maps(x, Wxh, b_xh, Whh, b_hh, Why, b_y)
    nc = _get_program()
    res = run_bass_kernel_spmd(nc, in_maps, list(range(NCORES)))
    _CACHE["last_results"] = res
    return np.concatenate([res.results[c]["out"] for c in range(NCORES)], axis=0)


def bench_floor(iters=20):
    """Per-exec wall time of a trivial 4-core kernel: the axon dispatch floor."""
    import time as _time
    from contextlib import ExitStack

    import jax
    from jax.sharding import Mesh, NamedSharding, PartitionSpec
    from jax.experimental.shard_map import shard_map
    import concourse.tile as tile
    from concourse import bacc, bass2jax, mybir

    nc = bacc.Bacc("TRN2", target_bir_lowering=False, debug=False, num_devices=NCORES)
    inp = nc.declare_dram_parameter("inp", [P, P], mybir.dt.float32, isOutput=False)
    outp = nc.declare_dram_parameter("out", [P, P], mybir.dt.float32, isOutput=True)
    with tile.TileContext(nc) as tc, ExitStack() as ctx:
        pool = ctx.enter_context(tc.tile_pool(name="p", bufs=1))
        tl = pool.tile([P, P], mybir.dt.float32, name="tl", tag="tl")
        nc.sync.dma_start(tl[:], inp[:, :])
        nc.sync.dma_start(outp[:, :], tl[:])
    nc.compile()
    bass2jax.install_neuronx_cc_hook()

    partition_name = nc.partition_id_tensor.name if nc.partition_id_tensor else None
    out_avals = [jax.core.ShapedArray((P, P), np.float32)]
    all_in_names = ["inp", "out"] + ([partition_name] if partition_name else [])

    def _body(*args):
        operands = list(args)
        if partition_name:
            operands.append(bass2jax.partition_id_tensor())
        return tuple(
            bass2jax._bass_exec_p.bind(
                *operands,
                out_avals=tuple(out_avals),
                in_names=tuple(all_in_names),
                out_names=("out",),
                lowering_input_output_aliases=(),
                sim_require_finite=True,
                sim_require_nnan=True,
                nc=nc,
            )
        )

    devices = jax.devices()[:NCORES]
    mesh = Mesh(np.asarray(devices), ("core",))
    spec = PartitionSpec("core")
    sharded = jax.jit(
        shard_map(_body, mesh=mesh, in_specs=(spec, spec), out_specs=(spec,),
                  check_rep=False),
        keep_unused=True,
    )
    xin = jax.device_put(np.zeros((NCORES * P, P), np.float32),
                         NamedSharding(mesh, spec))
    zin = jax.device_put(np.zeros((NCORES * P, P), np.float32),
                         NamedSharding(mesh, spec))
    o = sharded(xin, zin)
    jax.block_until_ready(o)
    times = []
    for _ in range(iters):
        t0 = _time.perf_counter()
        o = sharded(xin, zin)
        jax.block_until_ready(o)
        times.append((_time.perf_counter() - t0) * 1e9)
    return times


def bench_hw(inputs, iters=8):
    """Steady-state per-execution wall time with device-resident inputs.

    Replicates run_bass_via_pjrt's shard_map dispatch, but device_puts
    the inputs once and reuses one jitted executable, so repeated calls
    measure (HW exec + dispatch) rather than 256MB of host->device
    traffic.  Returns (best_ns, [per_iter_ns]), plus the outputs of the
    last iteration for checking.
    """
    import time as _time

    import jax
    from jax.sharding import Mesh, NamedSharding, PartitionSpec
    from jax.experimental.shard_map import shard_map
    from concourse import bass2jax, mybir

    in_maps = _make_in_maps(**inputs)
    nc = _get_program()
    bass2jax.install_neuronx_cc_hook()

    partition_name = (
        nc.partition_id_tensor.name if nc.partition_id_tensor else None
    )
    in_names, out_names, out_avals, zero_outs = [], [], [], []
    for alloc in nc.m.functions[0].allocations:
        if not isinstance(alloc, mybir.MemoryLocationSet):
            continue
        name = alloc.memorylocations[0].name
        if alloc.kind == "ExternalInput":
            if name != partition_name:
                in_names.append(name)
        elif alloc.kind == "ExternalOutput":
            out_names.append(name)
            np_dt = mybir.dt.np(alloc.dtype)
            out_avals.append(
                jax.core.ShapedArray(tuple(alloc.tensor_shape), np_dt)
            )
            zero_outs.append(np.zeros(tuple(alloc.tensor_shape), np_dt))
    n_params = len(in_names)
    all_in_names = in_names + out_names
    if partition_name is not None:
        all_in_names = all_in_names + [partition_name]

    def _body(*args):
        operands = list(args)
        if partition_name is not None:
            operands.append(bass2jax.partition_id_tensor())
        outs = bass2jax._bass_exec_p.bind(
            *operands,
            out_avals=tuple(out_avals),
            in_names=tuple(all_in_names),
            out_names=tuple(out_names),
            lowering_input_output_aliases=(),
            sim_require_finite=True,
            sim_require_nnan=True,
            nc=nc,
        )
        return tuple(outs)

    devices = jax.devices()[:NCORES]
    mesh = Mesh(np.asarray(devices), ("core",))
    spec = PartitionSpec("core")
    sharded = jax.jit(
        shard_map(
            _body,
            mesh=mesh,
            in_specs=(spec,) * (n_params + len(out_names)),
            out_specs=(spec,) * len(out_names),
            check_rep=False,
        ),
        keep_unused=True,
    )
    concat_in = [
        jax.device_put(
            np.concatenate([np.asarray(m[nm]) for m in in_maps], axis=0),
            NamedSharding(mesh, spec),
        )
        for nm in in_names
    ]
    concat_zero = [
        jax.device_put(
            np.zeros((NCORES * z.shape[0], *z.shape[1:]), z.dtype),
            NamedSharding(mesh, spec),
        )
        for z in zero_outs
    ]
    jax.block_until_ready(concat_in)

    # warmup (compile)
    outs = sharded(*concat_in, *concat_zero)
    jax.block_until_ready(outs)

    times = []
    for _ in range(iters):
        t0 = _time.perf_counter()
        outs = sharded(*concat_in, *concat_zero)
        jax.block_until_ready(outs)
        times.append((_time.perf_counter() - t0) * 1e9)
    result = np.concatenate(
        [
            np.asarray(outs[0]).reshape(NCORES, BS, NCLS)[c]
            for c in range(NCORES)
        ],
        axis=0,
    )
    return min(times), times, result



# revision 9
# speedup vs baseline: 1.8402x; 1.8402x over previous
"""Trainium2 Bass kernel for CustomRNN.

Reference computation (all fp32):
    xproj = einsum('bti,ih->bth', x, Wxh) + b_xh          # B,T,HID
    h_{t+1} = tanh(xproj[:,t] + h_t @ Whh + b_hh)         # scan over T
    out = h_T @ Why + b_y                                  # B,NC
    B, T, IN, HID, NC = 1024, 128, 512, 1024, 10

Strategy:
  - Batch-parallel over 8 NeuronCores (no cross-core comms).
  - Per core everything is kept TRANSPOSED: hT[k, b] with hidden on
    partitions, batch on the free dim.  Both the recurrence (Whh-tiles
    stationary) and the input projection (Wxh-tiles stationary)
    accumulate into the same PSUM tile [128j, BS] and the tanh
    produces hT_{t+1} tiles directly in the layout the next step
    consumes -> zero per-step transposes.
  - x is pre-transposed on the host to [T, IN, B] so the per-timestep
    slab DMAs straight into [i-partition, batch] layout.
  - Matmul operands are float16: the PE runs fp16 at full rate
    (1 cycle per moving row) at ANY moving-dim size, unlike float32r
    which needs N >= 256 and therefore capped the previous version at
    4 cores (N=256).  fp16 keeps 10 mantissa bits (vs e8m11's 11) and
    PSUM accumulation stays fp32, so accuracy degrades only ~2x
    (absmax-rel ~5e-4, measured on a CPU bit-sim) -- far inside the
    2e-2 gate.  With 8 cores x full rate, per-core PE work per step is
    12288 rows ~ 5.1us, ~2x faster than the 4-core fp32r version.
"""

import os
import numpy as np

B, T, IN, HID, NCLS = 1024, 128, 512, 1024, 10
P = 128
NCORES = int(os.environ.get("RNN_NCORES", "8"))
BS = B // NCORES                 # per-core batch (moving dim N)
JT = HID // P                    # output (M) tiles of h
KT = HID // P                    # contraction tiles over h
IT = IN // P                     # contraction tiles over x
T_STEPS = int(os.environ.get("RNN_T", str(T)))
REPEAT = int(os.environ.get("RNN_REPEAT", "1"))  # timing aid: loop recurrence R times

_CACHE = {}


def _build_nc():
    from concourse import bacc, mybir

    f32 = mybir.dt.float32
    f16 = mybir.dt.float16
    nc = bacc.Bacc(
        "TRN2", target_bir_lowering=False, debug=False, num_devices=NCORES
    )

    xT = nc.declare_dram_parameter("xT", [T_STEPS, IN, BS], f16, isOutput=False)
    wxh = nc.declare_dram_parameter("wxh", [IN, HID], f16, isOutput=False)
    whh = nc.declare_dram_parameter("whh", [HID, HID], f16, isOutput=False)
    bias = nc.declare_dram_parameter("bias", [P, JT], f32, isOutput=False)
    why = nc.declare_dram_parameter("why", [HID, NCLS], f16, isOutput=False)
    b_y = nc.declare_dram_parameter("b_y", [1, NCLS], f16, isOutput=False)
    ones = nc.declare_dram_parameter("ones", [1, P], f16, isOutput=False)
    out = nc.declare_dram_parameter("out", [BS, NCLS], f32, isOutput=True)
    return nc, (xT, wxh, whh, bias, why, b_y, ones, out)


def _emit(nc, tensors, repeat=1):
    from contextlib import ExitStack

    import concourse.bass as bass
    import concourse.tile as tile
    from concourse import mybir

    f32 = mybir.dt.float32
    f16 = mybir.dt.float16
    TANH = mybir.ActivationFunctionType.Tanh
    ts = bass.ts
    xT, wxh, whh, bias, why, b_y, ones, out = tensors

    with tile.TileContext(nc) as tc, ExitStack() as ctx:
        const = ctx.enter_context(tc.tile_pool(name="const", bufs=1))
        xpool = ctx.enter_context(tc.tile_pool(name="x", bufs=8))
        pspool = ctx.enter_context(tc.tile_pool(name="ps", bufs=8, space="PSUM"))
        opool = ctx.enter_context(tc.tile_pool(name="o", bufs=2))

        # --- persistent SBUF: weights, biases, h double buffer ---
        whh_sb = [const.tile([P, HID], f16, name=f"whh{k}", tag=f"whh{k}") for k in range(KT)]
        wxh_sb = [const.tile([P, HID], f16, name=f"wxh{i}", tag=f"wxh{i}") for i in range(IT)]
        bias_sb = const.tile([P, JT], f32, name="bias_sb", tag="bias")
        why_sb = [const.tile([P, NCLS], f16, name=f"why{k}", tag=f"why{k}") for k in range(KT)]
        by_sb = const.tile([1, NCLS], f16, name="by_sb", tag="by")
        ones_sb = const.tile([1, P], f16, name="ones_sb", tag="ones")
        h_sb = [
            [const.tile([P, BS], f16, name=f"h{p}_{k}", tag=f"h{p}_{k}") for k in range(KT)]
            for p in range(2)
        ]

        for k in range(KT):
            nc.sync.dma_start(whh_sb[k][:], whh[ts(k, P), :])
            nc.sync.dma_start(why_sb[k][:], why[ts(k, P), :])
        for i in range(IT):
            nc.sync.dma_start(wxh_sb[i][:], wxh[ts(i, P), :])
        nc.sync.dma_start(bias_sb[:], bias[:, :])
        nc.sync.dma_start(by_sb[:], b_y[:, :])
        nc.sync.dma_start(ones_sb[:], ones[:, :])

        # --- recurrence over timesteps ---
        rep_ctx = tc.For_i(0, repeat, 1) if repeat > 1 else None
        if rep_ctx is not None:
            rep_ctx.__enter__()
        for t in range(T_STEPS):
            xt = xpool.tile([P, IT * BS], f16, name="xt", tag="xt")
            for i in range(IT):
                nc.sync.dma_start(
                    xt[:, ts(i, BS)], xT[t, ts(i, P), :]
                )
            cur = h_sb[t % 2]
            nxt = h_sb[(t + 1) % 2]
            # Phase 1: all groups' input projections first — these have no
            # dependency on the previous step's tanh outputs, so they give
            # the PE a head start while ACT/DMA tails drain.
            ps_list = []
            for j in range(JT):
                # pad each accumulator to a full 2KB PSUM bank so the
                # ACT tanh read of group j never shares a bank with the
                # PE's in-flight writes for other groups
                ps = pspool.tile([P, 512], f32, name="ps", tag="ps")
                ps_list.append(ps)
                for i in range(IT):
                    nc.tensor.matmul(
                        ps[:, 0:BS],
                        wxh_sb[i][:, ts(j, P)],
                        xt[:, ts(i, BS)],
                        start=(i == 0),
                        stop=(t == 0 and i == IT - 1),
                    )
            # Phase 2: recurrence + tanh per group.
            for j in range(JT):
                ps = ps_list[j]
                if t > 0:
                    for k in range(KT):
                        nc.tensor.matmul(
                            ps[:, 0:BS],
                            whh_sb[k][:, ts(j, P)],
                            cur[k][:],
                            start=False,
                            stop=(k == KT - 1),
                        )
                nc.scalar.activation(
                    nxt[j][:], ps[:, 0:BS], TANH, bias=bias_sb[:, j : j + 1]
                )

        if rep_ctx is not None:
            rep_ctx.__exit__(None, None, None)

        # --- logits: out[b, n] = h_T[b, :] @ Why + b_y ---
        hT = h_sb[T_STEPS % 2]
        for bh in range(BS // P):
            ps = pspool.tile([P, NCLS], f32, name="ps", tag="ps")
            for k in range(KT):
                nc.tensor.matmul(
                    ps[:],
                    hT[k][:, ts(bh, P)],
                    why_sb[k][:],
                    start=(k == 0),
                    stop=False,
                )
            # broadcast-add b_y via a K=1 matmul: ones[1,P].T @ b_y[1,N]
            nc.tensor.matmul(
                ps[:], ones_sb[:], by_sb[:], start=False, stop=True
            )
            osb = opool.tile([P, NCLS], f32, name="osb", tag="osb")
            nc.vector.tensor_copy(osb[:], ps[:])
            nc.sync.dma_start(out[ts(bh, P), :], osb[:])


def _get_program(repeat=None):
    if repeat is None:
        repeat = REPEAT
    key = f"nc{repeat}"
    if key not in _CACHE:
        nc, tensors = _build_nc()
        _emit(nc, tensors, repeat=repeat)
        nc.compile()
        _CACHE[key] = nc
    return _CACHE[key]


def _make_in_maps(x, Wxh, b_xh, Whh, b_hh, Why, b_y):
    x = np.asarray(x, dtype=np.float32)
    Wxh_h = np.asarray(Wxh, np.float32).astype(np.float16)
    Whh_h = np.asarray(Whh, np.float32).astype(np.float16)
    Why_h = np.asarray(Why, np.float32).astype(np.float16)
    b_y_h = np.asarray(b_y, np.float32).astype(np.float16).reshape(1, NCLS)
    bias = (np.asarray(b_xh, np.float32) + np.asarray(b_hh, np.float32))
    bias2d = np.ascontiguousarray(bias.reshape(JT, P).T)
    ones_row = np.ones((1, P), dtype=np.float16)

    # host-side layout marshaling: [B,T,IN] -> [T,IN,B], shard batch
    xT_full = np.transpose(x[:, :T_STEPS, :], (1, 2, 0)).astype(np.float16)

    in_maps = []
    for c in range(NCORES):
        in_maps.append(
            {
                "xT": np.ascontiguousarray(xT_full[:, :, c * BS : (c + 1) * BS]),
                "wxh": Wxh_h,
                "whh": Whh_h,
                "bias": bias2d,
                "why": Why_h,
                "b_y": b_y_h,
                "ones": ones_row,
            }
        )
    return in_maps


def kernel(x, Wxh, b_xh, Whh, b_hh, Why, b_y):
    from concourse.bass_utils import run_bass_kernel_spmd

    in_maps = _make_in_maps(x, Wxh, b_xh, Whh, b_hh, Why, b_y)
    nc = _get_program()
    res = run_bass_kernel_spmd(nc, in_maps, list(range(NCORES)))
    _CACHE["last_results"] = res
    return np.concatenate([res.results[c]["out"] for c in range(NCORES)], axis=0)


def bench_floor(iters=20):
    """Per-exec wall time of a trivial kernel: the axon dispatch floor."""
    import time as _time
    from contextlib import ExitStack

    import jax
    from jax.sharding import Mesh, NamedSharding, PartitionSpec
    from jax.experimental.shard_map import shard_map
    import concourse.tile as tile
    from concourse import bacc, bass2jax, mybir

    nc = bacc.Bacc("TRN2", target_bir_lowering=False, debug=False, num_devices=NCORES)
    inp = nc.declare_dram_parameter("inp", [P, P], mybir.dt.float32, isOutput=False)
    outp = nc.declare_dram_parameter("out", [P, P], mybir.dt.float32, isOutput=True)
    with tile.TileContext(nc) as tc, ExitStack() as ctx:
        pool = ctx.enter_context(tc.tile_pool(name="p", bufs=1))
        tl = pool.tile([P, P], mybir.dt.float32, name="tl", tag="tl")
        nc.sync.dma_start(tl[:], inp[:, :])
        nc.sync.dma_start(outp[:, :], tl[:])
    nc.compile()
    bass2jax.install_neuronx_cc_hook()

    partition_name = nc.partition_id_tensor.name if nc.partition_id_tensor else None
    out_avals = [jax.core.ShapedArray((P, P), np.float32)]
    all_in_names = ["inp", "out"] + ([partition_name] if partition_name else [])

    def _body(*args):
        operands = list(args)
        if partition_name:
            operands.append(bass2jax.partition_id_tensor())
        return tuple(
            bass2jax._bass_exec_p.bind(
                *operands,
                out_avals=tuple(out_avals),
                in_names=tuple(all_in_names),
                out_names=("out",),
                lowering_input_output_aliases=(),
                sim_require_finite=True,
                sim_require_nnan=True,
                nc=nc,
            )
        )

    devices = jax.devices()[:NCORES]
    mesh = Mesh(np.asarray(devices), ("core",))
    spec = PartitionSpec("core")
    sharded = jax.jit(
        shard_map(_body, mesh=mesh, in_specs=(spec, spec), out_specs=(spec,),
                  check_rep=False),
        keep_unused=True,
    )
    xin = jax.device_put(np.zeros((NCORES * P, P), np.float32),
                         NamedSharding(mesh, spec))
    zin = jax.device_put(np.zeros((NCORES * P, P), np.float32),
                         NamedSharding(mesh, spec))
    o = sharded(xin, zin)
    jax.block_until_ready(o)
    times = []
    for _ in range(iters):
        t0 = _time.perf_counter()
        o = sharded(xin, zin)
        jax.block_until_ready(o)
        times.append((_time.perf_counter() - t0) * 1e9)
    return times


def bench_hw(inputs, iters=8, repeat=None):
    """Steady-state per-execution wall time with device-resident inputs.

    Replicates run_bass_via_pjrt's shard_map dispatch, but device_puts
    the inputs once and reuses one jitted executable, so repeated calls
    measure (HW exec + dispatch) rather than host->device traffic.
    Returns (best_ns, [per_iter_ns]), plus the outputs of the last
    iteration for checking.
    """
    import time as _time

    import jax
    from jax.sharding import Mesh, NamedSharding, PartitionSpec
    from jax.experimental.shard_map import shard_map
    from concourse import bass2jax, mybir

    in_maps = _make_in_maps(**inputs)
    nc = _get_program(repeat=repeat)
    bass2jax.install_neuronx_cc_hook()

    partition_name = (
        nc.partition_id_tensor.name if nc.partition_id_tensor else None
    )
    in_names, out_names, out_avals, zero_outs = [], [], [], []
    for alloc in nc.m.functions[0].allocations:
        if not isinstance(alloc, mybir.MemoryLocationSet):
            continue
        name = alloc.memorylocations[0].name
        if alloc.kind == "ExternalInput":
            if name != partition_name:
                in_names.append(name)
        elif alloc.kind == "ExternalOutput":
            out_names.append(name)
            np_dt = mybir.dt.np(alloc.dtype)
            out_avals.append(
                jax.core.ShapedArray(tuple(alloc.tensor_shape), np_dt)
            )
            zero_outs.append(np.zeros(tuple(alloc.tensor_shape), np_dt))
    n_params = len(in_names)
    all_in_names = in_names + out_names
    if partition_name is not None:
        all_in_names = all_in_names + [partition_name]

    def _body(*args):
        operands = list(args)
        if partition_name is not None:
            operands.append(bass2jax.partition_id_tensor())
        outs = bass2jax._bass_exec_p.bind(
            *operands,
            out_avals=tuple(out_avals),
            in_names=tuple(all_in_names),
            out_names=tuple(out_names),
            lowering_input_output_aliases=(),
            sim_require_finite=True,
            sim_require_nnan=True,
            nc=nc,
        )
        return tuple(outs)

    devices = jax.devices()[:NCORES]
    mesh = Mesh(np.asarray(devices), ("core",))
    spec = PartitionSpec("core")
    sharded = jax.jit(
        shard_map(
            _body,
            mesh=mesh,
            in_specs=(spec,) * (n_params + len(out_names)),
            out_specs=(spec,) * len(out_names),
            check_rep=False,
        ),
        keep_unused=True,
    )
    concat_in = [
        jax.device_put(
            np.concatenate([np.asarray(m[nm]) for m in in_maps], axis=0),
            NamedSharding(mesh, spec),
        )
        for nm in in_names
    ]
    concat_zero = [
        jax.device_put(
            np.zeros((NCORES * z.shape[0], *z.shape[1:]), z.dtype),
            NamedSharding(mesh, spec),
        )
        for z in zero_outs
    ]
    jax.block_until_ready(concat_in)

    # warmup (compile)
    outs = sharded(*concat_in, *concat_zero)
    jax.block_until_ready(outs)

    times = []
    for _ in range(iters):
        t0 = _time.perf_counter()
        outs = sharded(*concat_in, *concat_zero)
        jax.block_until_ready(outs)
        times.append((_time.perf_counter() - t0) * 1e9)
    result = np.concatenate(
        [
            np.asarray(outs[0]).reshape(NCORES, BS, NCLS)[c]
            for c in range(NCORES)
        ],
        axis=0,
    )
    return min(times), times, result
